# revision 1
# baseline (speedup 1.0000x reference)
"""Chamfer-distance (nn_CD_loss) Trainium2 kernel.

Computes reference:
    p1 = pixel2xyz(target), p2 = pixel2xyz(pred)   (N=16384 points each)
    D[i,j] = |p1_i|^2 + |p2_j|^2 - 2 p1_i.p2_j
    m12 = mean over valid i of min over valid j of D[i,j]
    m21 = mean over valid j of min over valid i of D[i,j]
    return m12 + m21

Strategy (8 NeuronCores, SPMD):
  Each core owns a 2048-row slice of each direction's distance matrix.
  The -2*p1.p2 inner products run on the PE at K=27 contraction built from an
  exact 3-way bf16 split of the fp32 coordinates (8 of 9 cross-product groups,
  dropping only lo*lo), plus 3 ones-rows carrying a 3-way bf16 split of the
  (validity-masked, +1e30) opposite-side squared norms.  PSUM tiles therefore
  hold E[i,j] = -2 p_i.q_j + sqq_masked[j] to ~1e-3 abs accuracy.
  Signs are folded so PSUM holds -E; the row-min becomes a row-MAX computed
  by the recurrence-free Max8 op: ScalarE stages every PSUM tile to SBUF
  (clean, ~1.0 ns/elem), DVE runs one nc.vector.max per tile (1011 ns /
  1024 elems, all 8 lanes kept so sort order is irrelevant) plus one
  reduce(max) per block.  Host computes dist = own_sq - max(-E) and the
  masked means (O(N) work).  This replaced the earlier tensor_tensor_scan
  pipeline (scan recurrence runs at ~1 result per 2 DVE cycles): measured
  580 us/rep vs the scan's 638-711 us in adjacent (hotter-device) sessions,
  with identical accuracy (rel err 1.72e-5).

  Measured on TRN2 silicon (on-device repeat-loop wall-clock deltas):
  ~550 us single-shot / ~600 us sustained per invocation; the Tile cost
  model predicts 317 us but does not model the scan's recurrence cost.
  Per-op DVE microbenchmarks (back-to-back, what actually binds):
    tensor_tensor_scan runs at ~1 result per 2 DVE cycles at every size
    (1305/2229/4457/8817 ns at F=512/1024/2048/4096 vs 691/1224/2291/4424
    streaming theory -- the serial state-feedback recurrence, proportional,
    no flat component to amortize).  Plain tensor_tensor is 1362 ns and
    tensor_reduce 1162 ns at F=1024 (both ~clean), and ScalarE PSUM copies
    are clean too (2004 ns at F=2048).  The scan still wins: single-pass
    reduction at 1.088 ns/element beats every clean-op alternative, since
    TT needs a second reduction pass over intermediates (>=1.18 ns/elem;
    elementwise TT trees do not shrink tile width, and the fp16 2x-mode
    tree only ties at ~1.06 with real precision risk).
  Variants measured and rejected on HW:
    - tensor_tensor_reduce: opcode is device-fatal on this runtime (NRT
      unrecoverable), hence the scan.
    - chunk=2048 (4-bank PSUM scan tiles, half the DVE ops): 749 us --
      PSUM only fits two such tiles so PE refills serialize against scans.
    - all-ScalarE-staged chained (sbuf,sbuf) scans: 697 us (chain links
      serialize through the scan ack latency).
    - deeper SBUF buffering (6,6): no change -- DVE is saturated, not
      ACT-stalled.
    - GPSIMD tensor_tensor(min) pre-combining: no such opcode on the Pool
      engine (walrus engine-check rejects); ScalarE accumulation is
      sum-only; fp16 2x-mode DVE trees save ~7% for real precision risk.
    - scan with stride-0 broadcast out (final state lands on one address,
      would drop the scratch tiles): correct on HW but 12% slower per op
      (2386 vs 2121 ns) -- the recurrence, not the write path, is the limit.
  The Max8 conversion above is the realization of that measured win; both
  DVE (~16.5 us/block) and ScalarE (~16.0 us/block) now run near-saturated.
"""

import numpy as np
import ml_dtypes

import concourse.bacc as bacc
import concourse.mybir as mybir
import concourse.tile as tile
from concourse.bass_utils import run_bass_kernel_spmd

H = W = 128
N = H * W                  # 16384 points per cloud
NCORES = 8
SHARE = N // NCORES        # 2048 rows per core per direction
BLOCKS = SHARE // 128      # 16 row-blocks of 128
K = 27                     # contraction: 8 product groups * 3 coords + 3 sq rows
CHUNK = 1024               # psum tile free size (2 banks)
PAIRS = N // (2 * CHUNK)   # TTR pair-iterations per row-block (8)
INF = np.float32(1.0e30)

_BF16 = ml_dtypes.bfloat16
# (lhs split level, rhs split level); 0=hi 1=mid 2=lo.  All 9 except (2,2).
_GROUPS = [(0, 0), (0, 1), (1, 0), (0, 2), (2, 0), (1, 1), (1, 2), (2, 1)]


def _pixel2xyz(depth, P):
    """depth [1,1,H,W] fp32 -> [N,3] fp32 (mirrors reference._pixel2xyz)."""
    d = depth[0, 0]
    px = np.broadcast_to(np.arange(W, dtype=np.float32)[None, :], (H, W))
    py = np.broadcast_to(np.arange(H, dtype=np.float32)[:, None], (H, W))
    c_u, c_v, f_u, f_v = P[0, 2], P[1, 2], P[0, 0], P[1, 1]
    x = (px * (d + P[2, 3]) - (c_u * d + P[0, 3])) / f_u
    y = (py * (d + P[2, 3]) - (c_v * d + P[1, 3])) / f_v
    return np.stack((x, y, d), axis=-1).reshape(-1, 3).astype(np.float32)


def _split3(v):
    """Exact 3-way bf16 split of fp32 array: v == h + m + l."""
    h = v.astype(_BF16)
    r = v - h.astype(np.float32)
    m = r.astype(_BF16)
    r2 = r - m.astype(np.float32)
    l = r2.astype(_BF16)
    return h, m, l


def _lhs_emb(Q):
    """Stationary-side embedding of point set Q [n,3] -> [K, n] bf16."""
    s = _split3(2.0 * Q)           # each [n,3]; sign flipped so PSUM = -E
    rows = [s[a][:, c] for (a, _) in _GROUPS for c in range(3)]
    rows += [np.full(Q.shape[0], -1.0, dtype=_BF16)] * 3
    return np.stack(rows, axis=0)  # [27, n]


def _rhs_emb(R, sq_masked):
    """Moving-side embedding of point set R [n,3] + masked |R|^2 -> [K, n] bf16."""
    t = _split3(R)
    u = _split3(sq_masked)
    rows = [t[b][:, c] for (_, b) in _GROUPS for c in range(3)]
    rows += [u[0], u[1], u[2]]
    return np.stack(rows, axis=0)  # [27, n]


def build_program(chunk=CHUNK, psum_bufs=4, copy_bufs=3, scan_bufs=3, reps=1):
    """Build + compile the SPMD single-core program (same NEFF on all 8 cores)."""
    pairs = N // (2 * chunk)
    nc = bacc.Bacc("TRN2", target_bir_lowering=False, debug=False,
                   num_devices=NCORES)
    f32 = mybir.dt.float32
    bf16 = mybir.dt.bfloat16

    lhsA = nc.dram_tensor("lhsA", [K, SHARE], bf16, kind="ExternalInput")
    rhsA = nc.dram_tensor("rhsA", [K, N], bf16, kind="ExternalInput")
    lhsB = nc.dram_tensor("lhsB", [K, SHARE], bf16, kind="ExternalInput")
    rhsB = nc.dram_tensor("rhsB", [K, N], bf16, kind="ExternalInput")
    outA = nc.dram_tensor("outA", [128, BLOCKS], f32, kind="ExternalOutput")
    outB = nc.dram_tensor("outB", [128, BLOCKS], f32, kind="ExternalOutput")

    with tile.TileContext(nc) as tc:
        with (
            tc.tile_pool(name="const", bufs=1) as cpool,
            tc.tile_pool(name="psum", bufs=psum_bufs, space="PSUM") as ppool,
            tc.tile_pool(name="copies", bufs=copy_bufs) as copool,
            tc.tile_pool(name="scans", bufs=scan_bufs) as apool,
            tc.tile_pool(name="gath", bufs=2) as gpool,
        ):
            lhsA_sb = cpool.tile([K, SHARE], bf16, tag="lhsA")
            rhsA_sb = cpool.tile([K, N], bf16, tag="rhsA")
            lhsB_sb = cpool.tile([K, SHARE], bf16, tag="lhsB")
            rhsB_sb = cpool.tile([K, N], bf16, tag="rhsB")
            minA = cpool.tile([128, BLOCKS], f32, tag="minA")
            minB = cpool.tile([128, BLOCKS], f32, tag="minB")
            nc.sync.dma_start(lhsA_sb[:], lhsA[:])
            for d0 in range(0, N, 4096):
                nc.sync.dma_start(rhsA_sb[:, d0:d0 + 4096],
                                  rhsA[:, d0:d0 + 4096])
            nc.sync.dma_start(lhsB_sb[:], lhsB[:])
            for d0 in range(0, N, 4096):
                nc.sync.dma_start(rhsB_sb[:, d0:d0 + 4096],
                                  rhsB[:, d0:d0 + 4096])

            import contextlib
            loop_ctx = (tc.For_i(0, reps, 1, hint_engines=(mybir.EngineType.PE,))
                        if reps > 1 else contextlib.nullcontext())
            with loop_ctx:
              for lhs_sb, rhs_sb, minbuf, out_dram in (
                (lhsA_sb, rhsA_sb, minA, outA),
                (lhsB_sb, rhsB_sb, minB, outB),
              ):
                  for b in range(BLOCKS):
                      lhs_blk = lhs_sb[:, b * 128:(b + 1) * 128]
                      # PSUM holds -E.  ScalarE stages every tile to SBUF;
                      # DVE runs one clean Max8 per tile (all 8 lanes kept,
                      # order-independent) and one reduce(max) per block.
                      ntiles = N // chunk
                      acc = gpool.tile([128, 8 * ntiles], f32, tag="acc")
                      for q in range(ntiles):
                          base = q * chunk
                          pe_t = ppool.tile([128, chunk], f32, tag="ps")
                          for g in range(chunk // 512):
                              c0 = base + g * 512
                              nc.tensor.matmul(
                                  pe_t[:, g * 512:(g + 1) * 512], lhs_blk,
                                  rhs_sb[:, c0:c0 + 512], start=True, stop=True)
                          sb_t = copool.tile([128, chunk], f32, tag="cp")
                          nc.scalar.copy(sb_t[:], pe_t[:])
                          nc.vector.max(out=acc[:, 8 * q:8 * q + 8], in_=sb_t[:])
                      nc.vector.tensor_reduce(
                          minbuf[:, b:b + 1], acc[:], axis=mybir.AxisListType.X,
                          op=mybir.AluOpType.max)
                  nc.sync.dma_start(out_dram[:], minbuf[:])
    nc.compile()
    return nc


def host_prep(pred, target, P_rect):
    pred = np.asarray(pred, dtype=np.float32)
    target = np.asarray(target, dtype=np.float32)
    P_rect = np.asarray(P_rect, dtype=np.float32)
    p1 = _pixel2xyz(target, P_rect)
    p2 = _pixel2xyz(pred, P_rect)
    valid = (target[0] > 0).reshape(-1)
    sq1 = np.sum(p1 * p1, axis=1).astype(np.float32)
    sq2 = np.sum(p2 * p2, axis=1).astype(np.float32)
    sq1m = np.where(valid, sq1, INF).astype(np.float32)
    sq2m = np.where(valid, sq2, INF).astype(np.float32)
    lhsA = np.ascontiguousarray(_lhs_emb(p1))      # rows = p1 points
    rhsA = np.ascontiguousarray(_rhs_emb(p2, sq2m))
    lhsB = np.ascontiguousarray(_lhs_emb(p2))      # rows = p2 points
    rhsB = np.ascontiguousarray(_rhs_emb(p1, sq1m))
    return p1, p2, valid, sq1, sq2, lhsA, rhsA, lhsB, rhsB


def finalize(results, valid, sq1, sq2):
    minA = np.concatenate(
        [np.asarray(results[c]["outA"]).T.reshape(-1) for c in range(NCORES)])
    minB = np.concatenate(
        [np.asarray(results[c]["outB"]).T.reshape(-1) for c in range(NCORES)])
    n = float(valid.sum())
    dist12 = sq1.astype(np.float64) - minA.astype(np.float64)
    dist21 = sq2.astype(np.float64) - minB.astype(np.float64)
    m12 = dist12[valid].sum() / n
    m21 = dist21[valid].sum() / n
    return np.asarray(np.float32(m12 + m21))


def kernel(pred, target, P_rect):
    p1, p2, valid, sq1, sq2, lhsA, rhsA, lhsB, rhsB = host_prep(
        pred, target, P_rect)
    nc = build_program()
    in_maps = []
    for c in range(NCORES):
        sl = slice(c * SHARE, (c + 1) * SHARE)
        in_maps.append({
            "lhsA": np.ascontiguousarray(lhsA[:, sl]),
            "rhsA": rhsA,
            "lhsB": np.ascontiguousarray(lhsB[:, sl]),
            "rhsB": rhsB,
        })
    try:
        res = run_bass_kernel_spmd(nc, in_maps, core_ids=list(range(NCORES)))
    except ModuleNotFoundError:
        # BASS_TRACE set but the axon NTFF hook is unavailable in this
        # environment; retry with tracing hard-disabled.
        import os
        os.environ["BASS_NEVER_TRACE"] = "1"
        res = run_bass_kernel_spmd(nc, in_maps, core_ids=list(range(NCORES)))
    return finalize(res.results, valid, sq1, sq2)



# revision 2
# speedup vs baseline: 1.9434x; 1.9434x over previous
"""Chamfer-distance (nn_CD_loss) Trainium2 kernel — single-pass D design.

Computes reference:
    p1 = pixel2xyz(target), p2 = pixel2xyz(pred)   (N=16384 points each)
    D[i,j] = |p1_i|^2 + |p2_j|^2 - 2 p1_i.p2_j
    m12 = mean over valid i of min over valid j of D[i,j]
    m21 = mean over valid j of min over valid i of D[i,j]
    return m12 + m21

Strategy (8 NeuronCores, SPMD), v2 — one D matrix, both reductions:
  Each core owns a 2048-row stripe of the SINGLE distance matrix
  (rows = its p1 slice, cols = all 16384 p2 points).  The GEMM carries
  BOTH squared-norm terms (K=30: 24 bf16-split product rows + 3 rows of
  -sq2m[j] against ones + 3 rows of -sq1m[i] against ones), so PSUM
  holds -D[i,j] <= 0 directly; near the row/col maxima the values are
  ~-dist (small), which makes bf16 staging precision-safe: only
  near-ties (within ~0.4%) can flip the argmax, changing the result by
  <0.4% against a 2e-2 gate.
  Per [128,2048] PSUM chunk:
    ACT stages fp32 -> SBUF bf16 (~0.92 ns/elem incl overhead).
    DVE (bf16 tensor ops run in 2x_1p mode, 0.52 ns/elem):
      rowacc[128,2048]  = max(rowacc, chunk)   (fold over the 8 chunks)
      colacc[:, chunk]  = max(colacc, chunk)   (fold over the 16 blocks)
    (chunk 0 of a block / block 0 of a column use tensor_copy, 4x mode,
     which also re-initializes the accumulators each repeat iteration.)
  Per block: one tensor_reduce(max) of rowacc -> -dist12 for 128 rows.
  End: colacc [128,16384] is transposed 128x128 at a time on the PE
  (bf16 transpose -> PSUM bf16), and DVE tensor_reduce(max) over the
  transposed free axis folds the partition direction -> per-column
  -min over this core's 2048 rows; host takes the max across cores.
  This does the whole job with ONE pass over D (the old kernel built
  D and D^T separately): PE work halves, and the reduce path drops from
  (1 ScalarE copy + 1 DVE fp32 max)/elem x 2 directions to
  (1 ACT copy + 2 bf16-2x DVE ops)/elem x 1 direction.
"""

import numpy as np
import ml_dtypes

import concourse.bacc as bacc
import concourse.mybir as mybir
import concourse.tile as tile
from concourse.bass_utils import run_bass_kernel_spmd

H = W = 128
N = H * W                  # 16384 points per cloud
NCORES = 8
SHARE = N // NCORES        # 2048 rows per core
BLOCKS = SHARE // 128      # 16 row-blocks of 128
K = 30                     # 24 product rows + 3 (-sq2m) rows + 3 (-sq1m) rows
CHUNK = 2048               # psum tile free size (4 banks)
NCHUNK = N // CHUNK        # 8 chunks per block row
INF = np.float32(1.0e30)

_BF16 = ml_dtypes.bfloat16
# (lhs split level, rhs split level); 0=hi 1=mid 2=lo.  All 9 except (2,2).
_GROUPS = [(0, 0), (0, 1), (1, 0), (0, 2), (2, 0), (1, 1), (1, 2), (2, 1)]


def _pixel2xyz(depth, P):
    """depth [1,1,H,W] fp32 -> [N,3] fp32 (mirrors reference._pixel2xyz)."""
    d = depth[0, 0]
    px = np.broadcast_to(np.arange(W, dtype=np.float32)[None, :], (H, W))
    py = np.broadcast_to(np.arange(H, dtype=np.float32)[:, None], (H, W))
    c_u, c_v, f_u, f_v = P[0, 2], P[1, 2], P[0, 0], P[1, 1]
    x = (px * (d + P[2, 3]) - (c_u * d + P[0, 3])) / f_u
    y = (py * (d + P[2, 3]) - (c_v * d + P[1, 3])) / f_v
    return np.stack((x, y, d), axis=-1).reshape(-1, 3).astype(np.float32)


def _split3(v):
    """Exact-ish 3-way bf16 split of fp32 array: v ~= h + m + l."""
    h = v.astype(_BF16)
    r = v - h.astype(np.float32)
    m = r.astype(_BF16)
    r2 = r - m.astype(np.float32)
    l = r2.astype(_BF16)
    return h, m, l


def _lhs_emb(Q, sq_masked):
    """Stationary-side embedding [K, n]: split3(2Q) products + ones + -sq1m."""
    s = _split3(2.0 * Q)                       # each [n,3]
    rows = [s[a][:, c] for (a, _) in _GROUPS for c in range(3)]
    rows += [np.full(Q.shape[0], -1.0, dtype=_BF16)] * 3   # pair with sq2m rows
    rows += list(_split3(sq_masked))                        # pair with ones rows
    return np.stack(rows, axis=0)              # [30, n]


def _rhs_emb(R, sq_masked):
    """Moving-side embedding [K, n]: split3(R) products + sq2m + ones."""
    t = _split3(R)
    rows = [t[b][:, c] for (_, b) in _GROUPS for c in range(3)]
    rows += list(_split3(sq_masked))
    rows += [np.full(R.shape[0], -1.0, dtype=_BF16)] * 3
    return np.stack(rows, axis=0)              # [30, n]


def build_program(reps=1):
    """Build + compile the SPMD single-core program (same NEFF on all 8 cores)."""
    nc = bacc.Bacc("TRN2", target_bir_lowering=False, debug=False,
                   num_devices=NCORES)
    f32 = mybir.dt.float32
    bf16 = mybir.dt.bfloat16
    A = mybir.AluOpType
    X = mybir.AxisListType.X

    lhs = nc.dram_tensor("lhs", [K, SHARE], bf16, kind="ExternalInput")
    rhs = nc.dram_tensor("rhs", [K, N], bf16, kind="ExternalInput")
    ident = nc.dram_tensor("ident", [128, 128], bf16, kind="ExternalInput")
    minrow = nc.dram_tensor("minrow", [128, BLOCKS], f32, kind="ExternalOutput")
    colout = nc.dram_tensor("colout", [128, 128], f32, kind="ExternalOutput")

    with tile.TileContext(nc) as tc:
        with (
            tc.tile_pool(name="const", bufs=1) as cpool,
            tc.tile_pool(name="psum", bufs=2, space="PSUM") as ppool,
            tc.tile_pool(name="staged", bufs=3) as spool,
            tc.tile_pool(name="rowacc", bufs=2) as rpool,
        ):
            lhs_sb = cpool.tile([K, SHARE], bf16, tag="lhs")
            rhs_sb = cpool.tile([K, N], bf16, tag="rhs")
            id_sb = cpool.tile([128, 128], bf16, tag="id")
            colacc = cpool.tile([128, N], bf16, tag="colacc")
            minrow_sb = cpool.tile([128, BLOCKS], f32, tag="minrow")
            colout_sb = cpool.tile([128, 128], f32, tag="colout")
            nc.sync.dma_start(lhs_sb[:], lhs[:])
            for d0 in range(0, N, 4096):
                nc.sync.dma_start(rhs_sb[:, d0:d0 + 4096],
                                  rhs[:, d0:d0 + 4096])
            nc.sync.dma_start(id_sb[:], ident[:])

            import contextlib
            loop_ctx = (tc.For_i(0, reps, 1, hint_engines=(mybir.EngineType.PE,))
                        if reps > 1 else contextlib.nullcontext())
            with loop_ctx:
                for b in range(BLOCKS):
                    lhs_blk = lhs_sb[:, b * 128:(b + 1) * 128]
                    rowacc = rpool.tile([128, CHUNK], bf16, tag="ra")
                    for q in range(NCHUNK):
                        base = q * CHUNK
                        pe_t = ppool.tile([128, CHUNK], f32, tag="ps")
                        for g in range(CHUNK // 512):
                            c0 = base + g * 512
                            nc.tensor.matmul(
                                pe_t[:, g * 512:(g + 1) * 512], lhs_blk,
                                rhs_sb[:, c0:c0 + 512], start=True, stop=True)
                        sb_t = spool.tile([128, CHUNK], bf16, tag="st")
                        nc.scalar.copy(sb_t[:], pe_t[:])
                        # row fold (over chunks) and col fold (over blocks)
                        if q == 0:
                            nc.vector.tensor_copy(rowacc[:], sb_t[:])
                        else:
                            nc.vector.tensor_tensor(
                                out=rowacc[:], in0=sb_t[:], in1=rowacc[:],
                                op=A.max)
                        cslice = colacc[:, base:base + CHUNK]
                        if b == 0:
                            nc.vector.tensor_copy(cslice, sb_t[:])
                        else:
                            nc.vector.tensor_tensor(
                                out=cslice, in0=sb_t[:], in1=cslice, op=A.max)
                    nc.vector.tensor_reduce(
                        minrow_sb[:, b:b + 1], rowacc[:], axis=X, op=A.max)
                # fold colacc's partition axis: PE-transpose 128 cols at a
                # time (bf16 stays bf16 in PSUM), reduce the free axis.
                for grp in range(16):
                    tr_t = ppool.tile([128, 8, 128], bf16, tag="ps")
                    for s in range(8):
                        j0 = grp * 1024 + s * 128
                        nc.tensor.transpose(
                            tr_t[:, s, :], colacc[:, j0:j0 + 128], id_sb[:])
                    nc.vector.tensor_reduce(
                        colout_sb[:, grp * 8:(grp + 1) * 8], tr_t[:],
                        axis=X, op=A.max)
                nc.sync.dma_start(minrow[:], minrow_sb[:])
                nc.sync.dma_start(colout[:], colout_sb[:])
    nc.compile()
    return nc


def host_prep(pred, target, P_rect):
    pred = np.asarray(pred, dtype=np.float32)
    target = np.asarray(target, dtype=np.float32)
    P_rect = np.asarray(P_rect, dtype=np.float32)
    p1 = _pixel2xyz(target, P_rect)
    p2 = _pixel2xyz(pred, P_rect)
    valid = (target[0] > 0).reshape(-1)
    sq1 = np.sum(p1 * p1, axis=1).astype(np.float32)
    sq2 = np.sum(p2 * p2, axis=1).astype(np.float32)
    sq1m = np.where(valid, sq1, INF).astype(np.float32)
    sq2m = np.where(valid, sq2, INF).astype(np.float32)
    lhs = np.ascontiguousarray(_lhs_emb(p1, sq1m))   # stationary: p1 rows
    rhs = np.ascontiguousarray(_rhs_emb(p2, sq2m))   # moving: all p2
    ident = np.eye(128, dtype=_BF16)
    return valid, lhs, rhs, ident


def make_in_maps(lhs, rhs, ident):
    in_maps = []
    for c in range(NCORES):
        sl = slice(c * SHARE, (c + 1) * SHARE)
        in_maps.append({
            "lhs": np.ascontiguousarray(lhs[:, sl]),
            "rhs": rhs,
            "ident": ident,
        })
    return in_maps


def finalize(results, valid):
    # minrow[c][p, b] = max_j -D[i,j] = -dist12[i],  i = c*2048 + b*128 + p
    dist12 = -np.concatenate(
        [np.asarray(results[c]["minrow"]).T.reshape(-1) for c in range(NCORES)]
    ).astype(np.float64)
    # colout[c][p, t] = max over core c's rows of -D[., j],  j = t*128 + p
    percore = np.stack(
        [np.asarray(results[c]["colout"]).T.reshape(-1) for c in range(NCORES)])
    dist21 = -percore.max(axis=0).astype(np.float64)
    n = float(valid.sum())
    m12 = dist12[valid].sum() / n
    m21 = dist21[valid].sum() / n
    return np.asarray(np.float32(m12 + m21))


def kernel(pred, target, P_rect):
    valid, lhs, rhs, ident = host_prep(pred, target, P_rect)
    nc = build_program()
    in_maps = make_in_maps(lhs, rhs, ident)
    try:
        res = run_bass_kernel_spmd(nc, in_maps, core_ids=list(range(NCORES)))
    except ModuleNotFoundError:
        # BASS_TRACE set but the axon NTFF hook is unavailable in this
        # environment; retry with tracing hard-disabled.
        import os
        os.environ["BASS_NEVER_TRACE"] = "1"
        res = run_bass_kernel_spmd(nc, in_maps, core_ids=list(range(NCORES)))
    return finalize(res.results, valid)


# revision 8
# speedup vs baseline: 2.1636x; 1.1133x over previous
"""Chamfer-distance (nn_CD_loss) Trainium2 kernel — single-pass D design.

Computes reference:
    p1 = pixel2xyz(target), p2 = pixel2xyz(pred)   (N=16384 points each)
    D[i,j] = |p1_i|^2 + |p2_j|^2 - 2 p1_i.p2_j
    m12 = mean over valid i of min over valid j of D[i,j]
    m21 = mean over valid j of min over valid i of D[i,j]
    return m12 + m21

Strategy (8 NeuronCores, SPMD), v2 — one D matrix, both reductions:
  Each core owns a 2048-row stripe of the SINGLE distance matrix
  (rows = its p1 slice, cols = all 16384 p2 points).  The GEMM carries
  BOTH squared-norm terms (K=30: 24 bf16-split product rows + 3 rows of
  -sq2m[j] against ones + 3 rows of -sq1m[i] against ones), so PSUM
  holds -D[i,j] <= 0 directly; near the row/col maxima the values are
  ~-dist (small), which makes bf16 staging precision-safe: only
  near-ties (within ~0.4%) can flip the argmax, changing the result by
  <0.4% against a 2e-2 gate.
  Per [128,2048] PSUM chunk:
    ACT stages fp32 -> SBUF bf16 (~0.92 ns/elem incl overhead).
    DVE (bf16 tensor ops run in 2x_1p mode, 0.52 ns/elem):
      rowacc[128,2048]  = max(rowacc, chunk)   (fold over the 8 chunks)
      colacc[:, chunk]  = max(colacc, chunk)   (fold over the 16 blocks)
    (chunk 0 of a block / block 0 of a column use tensor_copy, 4x mode,
     which also re-initializes the accumulators each repeat iteration.)
  Per block: one tensor_reduce(max) of rowacc -> -dist12 for 128 rows.
  End: colacc [128,16384] is transposed 128x128 at a time on the PE
  (bf16 transpose -> PSUM bf16), and DVE tensor_reduce(max) over the
  transposed free axis folds the partition direction -> per-column
  -min over this core's 2048 rows; host takes the max across cores.
  This does the whole job with ONE pass over D (the old kernel built
  D and D^T separately): PE work halves, and the reduce path drops from
  (1 ScalarE copy + 1 DVE fp32 max)/elem x 2 directions to
  (1 ACT copy + 2 bf16-2x DVE ops)/elem x 1 direction.
"""

import numpy as np
import ml_dtypes

import concourse.bacc as bacc
import concourse.bass_isa as bass_isa
import concourse.mybir as mybir
import concourse.tile as tile
from concourse.bass_utils import run_bass_kernel_spmd

H = W = 128
N = H * W                  # 16384 points per cloud
NCORES = 8
SHARE = N // NCORES        # 2048 rows per core
BLOCKS = SHARE // 128      # 16 row-blocks of 128
K = 30                     # 24 product rows + 3 (-sq2m) rows + 3 (-sq1m) rows
CHUNK = 2048               # psum tile free size (4 banks)
NCHUNK = N // CHUNK        # 8 chunks per block row
INF = np.float32(1.0e30)

_BF16 = ml_dtypes.bfloat16
# (lhs split level, rhs split level); 0=hi 1=mid 2=lo.  All 9 except (2,2).
_GROUPS = [(0, 0), (0, 1), (1, 0), (0, 2), (2, 0), (1, 1), (1, 2), (2, 1)]


def _pixel2xyz(depth, P):
    """depth [1,1,H,W] fp32 -> [N,3] fp32 (mirrors reference._pixel2xyz)."""
    d = depth[0, 0]
    px = np.broadcast_to(np.arange(W, dtype=np.float32)[None, :], (H, W))
    py = np.broadcast_to(np.arange(H, dtype=np.float32)[:, None], (H, W))
    c_u, c_v, f_u, f_v = P[0, 2], P[1, 2], P[0, 0], P[1, 1]
    x = (px * (d + P[2, 3]) - (c_u * d + P[0, 3])) / f_u
    y = (py * (d + P[2, 3]) - (c_v * d + P[1, 3])) / f_v
    return np.stack((x, y, d), axis=-1).reshape(-1, 3).astype(np.float32)


def _split3(v):
    """Exact-ish 3-way bf16 split of fp32 array: v ~= h + m + l."""
    h = v.astype(_BF16)
    r = v - h.astype(np.float32)
    m = r.astype(_BF16)
    r2 = r - m.astype(np.float32)
    l = r2.astype(_BF16)
    return h, m, l


def _lhs_emb(Q, sq_masked):
    """Stationary-side embedding [K, n]: split3(2Q) products + ones + -sq1m."""
    s = _split3(2.0 * Q)                       # each [n,3]
    rows = [s[a][:, c] for (a, _) in _GROUPS for c in range(3)]
    rows += [np.full(Q.shape[0], -1.0, dtype=_BF16)] * 3   # pair with sq2m rows
    rows += list(_split3(sq_masked))                        # pair with ones rows
    return np.stack(rows, axis=0)              # [30, n]


def _rhs_emb(R, sq_masked):
    """Moving-side embedding [K, n]: split3(R) products + sq2m + ones."""
    t = _split3(R)
    rows = [t[b][:, c] for (_, b) in _GROUPS for c in range(3)]
    rows += list(_split3(sq_masked))
    rows += [np.full(R.shape[0], -1.0, dtype=_BF16)] * 3
    return np.stack(rows, axis=0)              # [30, n]


def build_program(reps=1):
    """Build + compile the SPMD single-core program (same NEFF on all 8 cores)."""
    nc = bacc.Bacc("TRN2", target_bir_lowering=False, debug=False,
                   num_devices=NCORES)
    f32 = mybir.dt.float32
    bf16 = mybir.dt.bfloat16
    A = mybir.AluOpType
    X = mybir.AxisListType.X

    lhs = nc.dram_tensor("lhs", [K, SHARE], bf16, kind="ExternalInput")
    rhs = nc.dram_tensor("rhs", [K, N], bf16, kind="ExternalInput")
    minrow = nc.dram_tensor("minrow", [128, BLOCKS], f32, kind="ExternalOutput")
    colout = nc.dram_tensor("colout", [1, N], bf16, kind="ExternalOutput")

    with tile.TileContext(nc) as tc:
        with (
            tc.tile_pool(name="const", bufs=1) as cpool,
            tc.tile_pool(name="psum", bufs=2, space="PSUM") as ppool,
            tc.tile_pool(name="staged", bufs=3) as spool,
            tc.tile_pool(name="rowacc", bufs=2) as rpool,
        ):
            lhs_sb = cpool.tile([K, SHARE], bf16, tag="lhs")
            rhs_sb = cpool.tile([K, N], bf16, tag="rhs")
            colacc = cpool.tile([128, N], bf16, tag="colacc")
            colmin_sb = cpool.tile([128, N], bf16, tag="colmin")
            minrow_sb = cpool.tile([128, BLOCKS], f32, tag="minrow")
            nc.sync.dma_start(lhs_sb[:], lhs[:])
            for d0 in range(0, N, 4096):
                nc.sync.dma_start(rhs_sb[:, d0:d0 + 4096],
                                  rhs[:, d0:d0 + 4096])

            import contextlib
            loop_ctx = (tc.For_i(0, reps, 1, hint_engines=(mybir.EngineType.PE,))
                        if reps > 1 else contextlib.nullcontext())
            with loop_ctx:
                for b in range(BLOCKS):
                    lhs_blk = lhs_sb[:, b * 128:(b + 1) * 128]
                    rowacc = rpool.tile([128, CHUNK], bf16, tag="ra")
                    for q in range(NCHUNK):
                        base = q * CHUNK
                        pe_t = ppool.tile([128, CHUNK], f32, tag="ps")
                        for g in range(CHUNK // 512):
                            c0 = base + g * 512
                            nc.tensor.matmul(
                                pe_t[:, g * 512:(g + 1) * 512], lhs_blk,
                                rhs_sb[:, c0:c0 + 512], start=True, stop=True)
                        sb_t = spool.tile([128, CHUNK], bf16, tag="st")
                        nc.scalar.copy(sb_t[:], pe_t[:])
                        # row fold (over chunks) and col fold (over blocks)
                        if q == 0:
                            nc.vector.tensor_copy(rowacc[:], sb_t[:])
                        else:
                            nc.vector.tensor_tensor(
                                out=rowacc[:], in0=sb_t[:], in1=rowacc[:],
                                op=A.max)
                        cslice = colacc[:, base:base + CHUNK]
                        if b == 0:
                            nc.vector.tensor_copy(cslice, sb_t[:])
                        else:
                            nc.vector.tensor_tensor(
                                out=cslice, in0=sb_t[:], in1=cslice, op=A.max)
                    nc.vector.tensor_reduce(
                        minrow_sb[:, b:b + 1], rowacc[:], axis=X, op=A.max)
                # fold colacc's partition axis on the (otherwise idle) Pool
                # engine; the all-reduce broadcasts the result to every
                # partition, so DMA out row 0 only.
                nc.gpsimd.partition_all_reduce(
                    colmin_sb[:], colacc[:], channels=128,
                    reduce_op=bass_isa.ReduceOp.max)
                nc.sync.dma_start(minrow[:], minrow_sb[:])
                nc.sync.dma_start(colout[:], colmin_sb[0:1, :])
    nc.compile()
    return nc


def host_prep(pred, target, P_rect):
    pred = np.asarray(pred, dtype=np.float32)
    target = np.asarray(target, dtype=np.float32)
    P_rect = np.asarray(P_rect, dtype=np.float32)
    p1 = _pixel2xyz(target, P_rect)
    p2 = _pixel2xyz(pred, P_rect)
    valid = (target[0] > 0).reshape(-1)
    sq1 = np.sum(p1 * p1, axis=1).astype(np.float32)
    sq2 = np.sum(p2 * p2, axis=1).astype(np.float32)
    sq1m = np.where(valid, sq1, INF).astype(np.float32)
    sq2m = np.where(valid, sq2, INF).astype(np.float32)
    lhs = np.ascontiguousarray(_lhs_emb(p1, sq1m))   # stationary: p1 rows
    rhs = np.ascontiguousarray(_rhs_emb(p2, sq2m))   # moving: all p2
    return valid, lhs, rhs


def make_in_maps(lhs, rhs):
    in_maps = []
    for c in range(NCORES):
        sl = slice(c * SHARE, (c + 1) * SHARE)
        in_maps.append({
            "lhs": np.ascontiguousarray(lhs[:, sl]),
            "rhs": rhs,
        })
    return in_maps


def finalize(results, valid):
    # minrow[c][p, b] = max_j -D[i,j] = -dist12[i],  i = c*2048 + b*128 + p
    dist12 = -np.concatenate(
        [np.asarray(results[c]["minrow"]).T.reshape(-1) for c in range(NCORES)]
    ).astype(np.float64)
    # colout[c][0, j] = max over core c's rows of -D[., j]
    percore = np.stack(
        [np.asarray(results[c]["colout"]).reshape(-1).astype(np.float32)
         for c in range(NCORES)])
    dist21 = -percore.max(axis=0).astype(np.float64)
    n = float(valid.sum())
    m12 = dist12[valid].sum() / n
    m21 = dist21[valid].sum() / n
    return np.asarray(np.float32(m12 + m21))


def kernel(pred, target, P_rect):
    valid, lhs, rhs = host_prep(pred, target, P_rect)
    nc = build_program()
    in_maps = make_in_maps(lhs, rhs)
    try:
        res = run_bass_kernel_spmd(nc, in_maps, core_ids=list(range(NCORES)))
    except ModuleNotFoundError:
        # BASS_TRACE set but the axon NTFF hook is unavailable in this
        # environment; retry with tracing hard-disabled.
        import os
        os.environ["BASS_NEVER_TRACE"] = "1"
        res = run_bass_kernel_spmd(nc, in_maps, core_ids=list(range(NCORES)))
    return finalize(res.results, valid)
